# revision 16
# baseline (speedup 1.0000x reference)
"""Dense transformer block (B=4, T=2048, C=1024, H=16, FF=4096) on 8
Trainium2 NeuronCores.

Sharding: sequence-parallel, zero collectives. Core c handles batch
b = c // 2 and a zigzag set of 1024 query tokens (r = c % 2): 256-token
global blocks {0,3,4,7} for r=0, {1,2,5,6} for r=1. The host permutes
each core's token axis so its OWN query tokens occupy columns [0:1024)
(in block order) and the partner's tokens columns [1024:2048). K/V/LN1
are computed for all 2048 tokens (redundantly within a pair), so no
cross-core communication is needed. Causality is enforced with a
shared triangular diagonal mask plus per-core 0/1 block masks (input
data), which keeps the single SPMD program uniform across cores.

v2: all-bf16 data path (fp32 PSUM accumulation and statistics), no
DRAM bounces (K/V/Q/y stay in SBUF; odd-head y merges via SBUF->SBUF
DMA partition shift), 256-token q-blocks with exact kv-chunk coverage,
batched exp over 2-chunk score groups, ACT Reciprocal/Rsqrt for
softmax/LN, GPSIMD partition_broadcast for row broadcasts, and
attention interleaved with the first FFN half to keep TensorE dense.
"""
import numpy as np
import ml_dtypes

B, T, C = 4, 2048, 1024
H, D, FF = 16, 64, 4096
NC = 8
NKC = C // 128     # 8 feature chunks
NFFC = FF // 128   # 32
NVCH = T // 128    # 16 kv chunks
OWN = 1024         # own query tokens per core
EPS = 1e-5

_STATE = {}

# kv chunk production order: block b needs own chunks [0:2b+2) and
# partner chunks [8:8+2b+2)
VCH_ORDER = [0, 1, 8, 9, 2, 3, 10, 11, 4, 5, 12, 13, 6, 7, 14, 15]


def _build_program():
    import concourse.bacc as bacc
    import concourse.mybir as mybir
    from concourse.tile import TileContext

    F32 = mybir.dt.float32
    BF16 = mybir.dt.bfloat16
    AF = mybir.ActivationFunctionType
    OP = mybir.AluOpType

    nc = bacc.Bacc("TRN2", target_bir_lowering=False, debug=False,
                   num_devices=NC)

    xt_d = nc.dram_tensor("xt", [128, NKC, T], BF16, kind="ExternalInput")
    wq_d = nc.dram_tensor("wq", [8, 128, NKC, 128], BF16, kind="ExternalInput")
    wk_d = nc.dram_tensor("wk", [8, 128, NKC, 128], BF16, kind="ExternalInput")
    wv_d = nc.dram_tensor("wv", [2, 128, NKC, 512], BF16, kind="ExternalInput")
    wp_d = nc.dram_tensor("wp", [8, 128, NKC, 128], BF16, kind="ExternalInput")
    wf1_d = nc.dram_tensor("wf1", [NFFC, 128, NKC, 128], BF16,
                           kind="ExternalInput")
    wf2_d = nc.dram_tensor("wf2", [NKC, 128, NFFC, 128], BF16,
                           kind="ExternalInput")
    g1_d = nc.dram_tensor("g1", [128, NKC], F32, kind="ExternalInput")
    b1_d = nc.dram_tensor("b1", [128, NKC], F32, kind="ExternalInput")
    g2_d = nc.dram_tensor("g2", [128, NKC], F32, kind="ExternalInput")
    b2_d = nc.dram_tensor("b2", [128, NKC], F32, kind="ExternalInput")
    bp_d = nc.dram_tensor("bp", [128, NKC], F32, kind="ExternalInput")
    bf1_d = nc.dram_tensor("bf1", [128, NFFC], F32, kind="ExternalInput")
    bf2_d = nc.dram_tensor("bf2", [128, NKC], F32, kind="ExternalInput")
    # triangular diagonal mask (same on all cores): [128 kv, 2 chunks, 256 q]
    mtri_d = nc.dram_tensor("mtri", [128, 2, 256], BF16, kind="ExternalInput")
    # per-block partner-pair mask value (0.0 or 1.0), per core
    mdep_d = nc.dram_tensor("mdep", [128, 4], F32, kind="ExternalInput")
    out_d = nc.dram_tensor("out", [128, NKC, OWN], F32, kind="ExternalOutput")

    def mm(ps, lhsT, rhs, start, stop, **kw):
        nc.tensor.matmul(ps, lhsT, rhs, start=start, stop=stop, **kw)

    with TileContext(nc, pool_alloc_mode="queue") as tc:
        consts_cm = tc.tile_pool(name="consts", bufs=1)
        consts = consts_cm.__enter__()

        ones128 = consts.tile([128, 1], BF16)
        nc.vector.memset(ones128, 1.0)
        eps_t = consts.tile([1, 1], F32)
        nc.vector.memset(eps_t, EPS)
        g1t = consts.tile([128, NKC], F32)
        nc.sync.dma_start(out=g1t, in_=g1_d[:, :])
        b1t = consts.tile([128, NKC], F32)
        nc.sync.dma_start(out=b1t, in_=b1_d[:, :])
        g2t = consts.tile([128, NKC], F32)
        nc.sync.dma_start(out=g2t, in_=g2_d[:, :])
        b2t = consts.tile([128, NKC], F32)
        nc.sync.dma_start(out=b2t, in_=b2_d[:, :])
        bpt = consts.tile([128, NKC], F32)
        nc.sync.dma_start(out=bpt, in_=bp_d[:, :])
        bf1t = consts.tile([128, NFFC], F32)
        nc.sync.dma_start(out=bf1t, in_=bf1_d[:, :])
        bf2t = consts.tile([128, NKC], F32)
        nc.sync.dma_start(out=bf2t, in_=bf2_d[:, :])
        mtri = consts.tile([128, 2, 256], BF16)
        nc.sync.dma_start(out=mtri, in_=mtri_d[:, :, :])
        mdep = consts.tile([128, 4], F32)
        nc.sync.dma_start(out=mdep, in_=mdep_d[:, :])

        # ---------------- layer norm over feature dim (bf16) -------------
        def layer_norm(src, dst, gt, bt, psum, work):
            """src [128, NKC, 512] bf16 view; dst same-shape bf16 view.
            Stats via PE ones-matmuls, rstd = exp(-0.5*ln(var+eps)) on ACT
            (Ln/Exp share one table set with attention Exp), broadcasts on
            GPSIMD, normalize on DVE."""
            ps_s = psum.tile([128, 512], F32, tag="mm")
            for k in range(NKC):
                mm(ps_s[0:1, :], ones128, src[:, k, :], k == 0, k == NKC - 1)
            sqs = []
            for k in range(NKC):
                sq = work.tile([128, 512], BF16, tag="sq", bufs=3)
                nc.scalar.activation(out=sq, in_=src[:, k, :], func=AF.Square)
                sqs.append(sq)
            ps_q = psum.tile([128, 512], F32, tag="mm")
            for k in range(NKC):
                mm(ps_q[0:1, :], ones128, sqs[k], k == 0, k == NKC - 1)
            mu_f = work.tile([1, 512], F32, tag="st", bufs=4)
            nc.vector.tensor_scalar_mul(out=mu_f, in0=ps_s[0:1, :],
                                        scalar1=1.0 / C)
            mu2 = work.tile([1, 512], F32, tag="st", bufs=4)
            nc.vector.tensor_mul(out=mu2, in0=mu_f, in1=mu_f)
            var = work.tile([1, 512], F32, tag="st", bufs=4)
            nc.vector.scalar_tensor_tensor(
                out=var, in0=ps_q[0:1, :], scalar=1.0 / C, in1=mu2,
                op0=OP.mult, op1=OP.subtract)
            lnv = work.tile([1, 512], F32, tag="st", bufs=4)
            nc.scalar.activation(out=lnv, in_=var, func=AF.Ln,
                                 bias=eps_t, scale=1.0)
            rstd = work.tile([1, 512], BF16, tag="sv", bufs=3)
            with nc.allow_low_precision(reason="bf16 rstd"):
                nc.scalar.activation(out=rstd, in_=lnv, func=AF.Exp,
                                     scale=-0.5)
            mu_bf = work.tile([1, 512], BF16, tag="sv", bufs=3)
            nc.vector.tensor_copy(out=mu_bf, in_=mu_f)
            mu_b = work.tile([128, 512], BF16, tag="bc", bufs=3)
            nc.gpsimd.partition_broadcast(mu_b, mu_bf)
            rs_b = work.tile([128, 512], BF16, tag="bc", bufs=3)
            nc.gpsimd.partition_broadcast(rs_b, rstd)
            for k in range(NKC):
                t1 = work.tile([128, 512], BF16, tag="tt", bufs=4)
                nc.vector.tensor_sub(out=t1, in0=src[:, k, :], in1=mu_b)
                t2 = work.tile([128, 512], BF16, tag="tt", bufs=4)
                nc.vector.tensor_mul(out=t2, in0=t1, in1=rs_b)
                nc.vector.tensor_scalar(
                    out=dst[:, k, :], in0=t2,
                    scalar1=gt[:, k:k + 1], scalar2=bt[:, k:k + 1],
                    op0=OP.mult, op1=OP.add)

        # ======== Phase 1: LN1 over all 2048 tokens ========
        big_cm = tc.tile_pool(name="big", bufs=1)
        big = big_cm.__enter__()
        lnw_cm = tc.tile_pool(name="lnw", bufs=1)
        lnw = lnw_cm.__enter__()
        lnx_cm = tc.tile_pool(name="lnxp", bufs=1)
        lnxp = lnx_cm.__enter__()
        lnx = lnxp.tile([128, NKC, T], BF16)

        xt_cm = tc.tile_pool(name="xtp", bufs=2)
        xtp = xt_cm.__enter__()
        ps1_cm = tc.tile_pool(name="ln1ps", bufs=4, space="PSUM")
        ps1 = ps1_cm.__enter__()
        for tb in range(4):
            sl = slice(tb * 512, (tb + 1) * 512)
            xtb = xtp.tile([128, NKC, 512], BF16, tag="xtb")
            nc.sync.dma_start(out=xtb, in_=xt_d[:, :, sl])
            layer_norm(xtb, lnx[:, :, sl], g1t, b1t, ps1, lnw)
        ps1_cm.__exit__(None, None, None)
        xt_cm.__exit__(None, None, None)

        # ======== Phase 2: QKV projections (SBUF-resident outputs) =======
        qt = big.tile([128, NKC, OWN], BF16, tag="q")
        kt = big.tile([128, NKC, T], BF16, tag="k")
        vt = big.tile([128, NVCH, H, 65], BF16, tag="v")
        nc.vector.memset(vt[:, :, :, 64:65], 1.0)

        qw_cm = tc.tile_pool(name="qkvw", bufs=3)
        qw = qw_cm.__enter__()
        wv_cm = tc.tile_pool(name="wvp", bufs=2)
        wvp = wv_cm.__enter__()
        psq_cm = tc.tile_pool(name="qkvps", bufs=4, space="PSUM")
        psq = psq_cm.__enter__()

        # Q (own tokens only; 1/sqrt(D) folded into wq host-side)
        for oc in range(8):
            wt = qw.tile([128, NKC, 128], BF16, tag="w")
            nc.sync.dma_start(out=wt, in_=wq_d[oc])
            for tb in range(2):
                sl = slice(tb * 512, (tb + 1) * 512)
                ps = psq.tile([128, 512], F32, tag="mm")
                for k in range(NKC):
                    mm(ps, wt[:, k, :], lnx[:, k, sl], k == 0, k == NKC - 1)
                nc.scalar.activation(out=qt[:, oc, sl], in_=ps, func=AF.Copy)
        # K (all tokens)
        for oc in range(8):
            wt = qw.tile([128, NKC, 128], BF16, tag="w")
            nc.sync.dma_start(out=wt, in_=wk_d[oc])
            for tb in range(4):
                sl = slice(tb * 512, (tb + 1) * 512)
                ps = psq.tile([128, 512], F32, tag="mm")
                for k in range(NKC):
                    mm(ps, wt[:, k, :], lnx[:, k, sl], k == 0, k == NKC - 1)
                nc.scalar.activation(out=kt[:, oc, sl], in_=ps, func=AF.Copy)
        # V (all tokens; natural [token, feature] layout, chunk order by
        # attention need)
        wvt = []
        for g in range(2):
            wvg = wvp.tile([128, NKC, 512], BF16, tag="wv")
            nc.sync.dma_start(out=wvg, in_=wv_d[g])
            wvt.append(wvg)
        for cch in VCH_ORDER:
            for g in range(2):
                ps = psq.tile([128, 512], F32, tag="mm")
                for k in range(NKC):
                    mm(ps, lnx[:, k, cch * 128:(cch + 1) * 128], wvt[g][:, k, :],
                       k == 0, k == NKC - 1)
                nc.scalar.activation(
                    out=vt[:, cch, g * 8:(g + 1) * 8, 0:64], in_=ps,
                    func=AF.Copy)
        psq_cm.__exit__(None, None, None)
        wv_cm.__exit__(None, None, None)
        qw_cm.__exit__(None, None, None)
        lnx_cm.__exit__(None, None, None)

        # ======== pooled tiles for the back half (opened before the
        # attention pools so the attention pools can close mid-stream
        # in LIFO order) ========
        act_cm = tc.tile_pool(name="actp", bufs=1)
        actp = act_cm.__enter__()
        ytp = actp
        x2p = actp
        ln2p = actp
        m1_cm = tc.tile_pool(name="m1p", bufs=1)
        m1p = m1_cm.__enter__()
        tail_cm = tc.tile_pool(name="tailp", bufs=1)
        tailp = tail_cm.__enter__()
        gps_cm = tc.tile_pool(name="gps", bufs=2, space="PSUM")
        gps = gps_cm.__enter__()

        # ======== attention machinery ========
        attw_cm = tc.tile_pool(name="attw", bufs=1)
        attw = attw_cm.__enter__()
        ptmp = attw
        sps_cm = tc.tile_pool(name="attps", bufs=2, space="PSUM")
        sps = sps_cm.__enter__()
        yps_cm = tc.tile_pool(name="attpy", bufs=2, space="PSUM")
        yps = yps_cm.__enter__()

        yt_tiles = {}

        def attn(hp, b):
            """Head pair hp (heads 2hp, 2hp+1), own q block b (256 toks)."""
            qsl = slice(b * 256, (b + 1) * 256)
            chunks = list(range(0, 2 * b + 2)) + list(range(8, 8 + 2 * b + 2))
            groups = [(chunks[2 * i], chunks[2 * i + 1])
                      for i in range(len(chunks) // 2)]
            ng = len(groups)
            yt = yt_tiles[b // 2]
            bsl = slice((b % 2) * 256, (b % 2) * 256 + 256)
            ps_y = yps.tile([65, 2, 256], F32, tag="y")
            first = True
            for gi, (c0, c1) in enumerate(groups):
                ps_s = sps.tile([128, 2, 2, 256], F32, tag="s")
                for ji, ch in enumerate((c0, c1)):
                    csl = slice(ch * 128, (ch + 1) * 128)
                    mm(ps_s[:, ji, 0, :], kt[0:64, hp, csl],
                       qt[0:64, hp, qsl], True, True)
                    mm(ps_s[:, ji, 1, :], kt[64:128, hp, csl],
                       qt[64:128, hp, qsl], True, True)
                ptm = ptmp.tile([128, 2, 2, 256], BF16, tag="ptm", bufs=3)
                nc.scalar.activation(out=ptm, in_=ps_s, func=AF.Exp)
                if c0 == 2 * b:        # diagonal own pair -> triangular mask
                    for ji in range(2):
                        nc.vector.tensor_mul(
                            out=ptm[:, ji], in0=ptm[:, ji],
                            in1=mtri[:, ji:ji + 1, :]
                            .broadcast_to([128, 2, 256]))
                elif c0 == 8 + 2 * b:  # partner diagonal pair -> 0/1 scalar
                    nc.vector.tensor_scalar_mul(
                        out=ptm, in0=ptm, scalar1=mdep[:, b:b + 1])
                for ji, ch in enumerate((c0, c1)):
                    for h in range(2):
                        mm(ps_y[:, h, :], vt[:, ch, 2 * hp + h, 0:65],
                           ptm[:, ji, h, :], first,
                           (gi == ng - 1) and (ji == 1) and (h == 1),
                           skip_group_check=True)
                        first = False
            # softmax tail: reciprocal of denominators, broadcast, scale
            rd = attw.tile([1, 2, 256], F32, tag="rd", bufs=2)
            nc.vector.reciprocal_approx_fast(out=rd, in_=ps_y[64:65, :, :])
            rb = attw.tile([64, 2, 256], F32, tag="rb", bufs=2)
            nc.gpsimd.partition_broadcast(rb, rd)
            nc.vector.tensor_mul(out=yt[0:64, hp, bsl],
                                 in0=ps_y[0:64, 0, :], in1=rb[:, 0, :])
            ytmp = attw.tile([64, 256], BF16, tag="ytmp", bufs=2)
            nc.vector.tensor_mul(out=ytmp, in0=ps_y[0:64, 1, :],
                                 in1=rb[:, 1, :])
            nc.sync.dma_start(out=yt[64:128, hp, bsl], in_=ytmp)

        x2_tiles = {}
        ln2_tiles = {}
        m1_tiles = {}

        def proj_ln2(h):
            """proj + residual + LN2 for own-token half h (512 tokens)."""
            hsl = slice(h * 512, (h + 1) * 512)
            yt = yt_tiles[h]
            x2t = x2p.tile([128, NKC, 512], BF16, tag="x2", name=f"x2h{h}", bufs=2)
            x2_tiles[h] = x2t
            for oc in range(8):
                wt = tailp.tile([128, NKC, 128], BF16, tag="w", bufs=3)
                nc.sync.dma_start(out=wt, in_=wp_d[oc])
                xq = tailp.tile([128, 512], BF16, tag="xq", bufs=3)
                nc.sync.dma_start(out=xq, in_=xt_d[:, oc, hsl])
                ps = gps.tile([128, 512], F32, tag="mm")
                for k in range(NKC):
                    mm(ps, wt[:, k, :], yt[:, k, :], k == 0, k == NKC - 1)
                nc.vector.scalar_tensor_tensor(
                    out=x2t[:, oc, :], in0=ps, scalar=bpt[:, oc:oc + 1],
                    in1=xq, op0=OP.add, op1=OP.add)
            ln2t = ln2p.tile([128, NKC, 512], BF16, tag="ln2", name=f"ln2h{h}", bufs=1)
            ln2_tiles[h] = ln2t
            layer_norm(x2t, ln2t, g2t, b2t, gps, lnw)

        def fc1(h, ffc):
            wt = tailp.tile([128, NKC, 128], BF16, tag="w", bufs=3)
            nc.sync.dma_start(out=wt, in_=wf1_d[ffc])
            ps = gps.tile([128, 512], F32, tag="mm")
            for k in range(NKC):
                mm(ps, wt[:, k, :], ln2_tiles[h][:, k, :], k == 0, k == NKC - 1)
            nc.vector.tensor_scalar(
                out=m1_tiles[h][:, ffc, :], in0=ps,
                scalar1=bf1t[:, ffc:ffc + 1], scalar2=0.0,
                op0=OP.add, op1=OP.max)

        def fc2(h, oc, psum):
            hsl = slice(h * 512, (h + 1) * 512)
            ps = psum.tile([128, 512], F32, tag="mm")
            for half in range(2):
                wt2 = tailp.tile([128, 16, 128], BF16, tag="w2", bufs=2)
                nc.sync.dma_start(out=wt2,
                                  in_=wf2_d[oc][:, half * 16:half * 16 + 16, :])
                for j in range(16):
                    k = half * 16 + j
                    mm(ps, wt2[:, j, :], m1_tiles[h][:, k, :],
                       k == 0, k == NFFC - 1)
            ot = tailp.tile([128, 512], F32, tag="ot", bufs=2)
            nc.vector.scalar_tensor_tensor(
                out=ot, in0=ps, scalar=bf2t[:, oc:oc + 1],
                in1=x2_tiles[h][:, oc, :], op0=OP.add, op1=OP.add)
            nc.sync.dma_start(out=out_d[:, oc, hsl], in_=ot)

        # ======== Phase 3: attention blocks 0,1 then proj/LN2 half 0 =====
        yt_tiles[0] = ytp.tile([128, NKC, 512], BF16, tag="yt", name="yt0", bufs=1)
        for hp in range(8):
            attn(hp, 0)
            attn(hp, 1)
        proj_ln2(0)
        m1_tiles[0] = m1p.tile([128, NFFC, 512], BF16, tag="m1", name="m1h0")

        # ======== Phase 5: attention blocks 2,3 interleaved with FC1(h0) ==
        yt_tiles[1] = ytp.tile([128, NKC, 512], BF16, tag="yt", name="yt1", bufs=1)
        for hp in range(8):
            attn(hp, 2)
            fc1(0, 4 * hp + 0)
            fc1(0, 4 * hp + 1)
            attn(hp, 3)
            fc1(0, 4 * hp + 2)
            fc1(0, 4 * hp + 3)

        # ======== Phase 6: FC2 half 0; proj/LN2 half 1 ========
        for oc in range(4):
            fc2(0, oc, gps)
        proj_ln2(1)
        for oc in range(4, 8):
            fc2(0, oc, gps)

        # attention pools closed -> free PSUM for the tail
        yps_cm.__exit__(None, None, None)
        sps_cm.__exit__(None, None, None)
        attw_cm.__exit__(None, None, None)

        tl_cm = tc.tile_pool(name="tailps", bufs=4, space="PSUM")
        tlps = tl_cm.__enter__()

        # ======== Phase 7: FFN half 1 ========
        m1_tiles[1] = m1p.tile([128, NFFC, 512], BF16, tag="m1", name="m1h1")
        for ffc in range(NFFC):
            fc1(1, ffc)
        for oc in range(8):
            fc2(1, oc, tlps)

        tl_cm.__exit__(None, None, None)
        gps_cm.__exit__(None, None, None)
        tail_cm.__exit__(None, None, None)
        m1_cm.__exit__(None, None, None)
        act_cm.__exit__(None, None, None)
        lnw_cm.__exit__(None, None, None)
        big_cm.__exit__(None, None, None)
        consts_cm.__exit__(None, None, None)

    nc.compile()
    return nc


class _SpmdRunner:
    def __init__(self, nc, n_cores=NC):
        import jax
        from jax.sharding import Mesh, PartitionSpec
        from jax.experimental.shard_map import shard_map
        import concourse.mybir as mybir
        from concourse import bass2jax
        bass2jax.install_neuronx_cc_hook()
        self.jax = jax
        self.n_cores = n_cores
        partition_name = (
            nc.partition_id_tensor.name if nc.partition_id_tensor else None)
        in_names, out_names, out_avals = [], [], []
        for alloc in nc.m.functions[0].allocations:
            if not isinstance(alloc, mybir.MemoryLocationSet):
                continue
            name = alloc.memorylocations[0].name
            if alloc.kind == "ExternalInput":
                if name != partition_name:
                    in_names.append(name)
            elif alloc.kind == "ExternalOutput":
                out_names.append(name)
                out_avals.append(jax.core.ShapedArray(
                    tuple(alloc.tensor_shape), mybir.dt.np(alloc.dtype)))
        self.in_names = in_names
        self.out_names = out_names
        self.out_avals = out_avals
        all_in = in_names + out_names
        if partition_name is not None:
            all_in.append(partition_name)

        def _body(*args):
            operands = list(args)
            if partition_name is not None:
                operands.append(bass2jax.partition_id_tensor())
            outs = bass2jax._bass_exec_p.bind(
                *operands, out_avals=tuple(out_avals),
                in_names=tuple(all_in), out_names=tuple(out_names),
                lowering_input_output_aliases=(),
                sim_require_finite=True, sim_require_nnan=True, nc=nc)
            return tuple(outs)

        import os as _os
        if _os.environ.get("BASS_SIM") == "1":
            devices = jax.devices("cpu")[:n_cores]
        else:
            devices = jax.devices()[:n_cores]
        self.mesh = Mesh(np.asarray(devices), ("core",))
        n_io = len(in_names) + len(out_names)
        self.fn = jax.jit(
            shard_map(_body, mesh=self.mesh,
                      in_specs=(PartitionSpec("core"),) * n_io,
                      out_specs=(PartitionSpec("core"),) * len(out_names),
                      check_rep=False),
            keep_unused=True)
        self._dev_in = None

    def put_inputs(self, in_maps):
        from jax.sharding import NamedSharding, PartitionSpec
        jax = self.jax
        sh = NamedSharding(self.mesh, PartitionSpec("core"))
        concat = []
        for name in self.in_names:
            arrs = [np.asarray(in_maps[c][name]) for c in range(self.n_cores)]
            concat.append(jax.device_put(np.concatenate(arrs, axis=0), sh))
        for av in self.out_avals:
            z = np.zeros((self.n_cores * av.shape[0], *av.shape[1:]), av.dtype)
            concat.append(jax.device_put(z, sh))
        self._dev_in = concat

    def run(self):
        jax = self.jax
        outs = self.fn(*self._dev_in)
        jax.block_until_ready(outs)
        results = []
        for c in range(self.n_cores):
            d = {}
            for i, name in enumerate(self.out_names):
                av = self.out_avals[i]
                d[name] = np.asarray(outs[i]).reshape(
                    self.n_cores, *av.shape)[c]
            results.append(d)
        return results

    def time_exec(self, warmup=3, m1=4, m2=12, reps=3, trials=6):
        """Estimate per-call device time by dispatching bursts of m1 and
        m2 back-to-back calls and differencing, which cancels the
        constant dispatch/RTT overhead of the axon tunnel."""
        import time
        jax = self.jax
        for _ in range(warmup):
            jax.block_until_ready(self.fn(*self._dev_in))

        def burst(m):
            t0 = time.perf_counter()
            outs = None
            for _ in range(m):
                outs = self.fn(*self._dev_in)
            jax.block_until_ready(outs)
            return time.perf_counter() - t0

        t1s, t2s = [], []
        for _ in range(trials):
            for _ in range(reps):
                t1s.append(burst(m1))
                t2s.append(burst(m2))
        return (min(t2s) - min(t1s)) / (m2 - m1)


def _get_runner():
    if "runner" not in _STATE:
        nc = _build_program()
        _STATE["runner"] = _SpmdRunner(nc)
    return _STATE["runner"]


def _q_token_sel(r):
    """256-token global blocks {0,3,4,7} for r=0, {1,2,5,6} for r=1."""
    if r == 0:
        return np.concatenate([np.arange(0, 256), np.arange(768, 1280),
                               np.arange(1792, 2048)])
    return np.concatenate([np.arange(256, 768), np.arange(1280, 1792)])


def _core_token_order(r):
    own = _q_token_sel(r)
    partner = np.setdiff1d(np.arange(T), own)
    return np.concatenate([own, partner])


def _prep_in_maps(x, W_attn, W_proj, b_proj, W_fc1, b_fc1, W_fc2, b_fc2,
                  ln1_g, ln1_b, ln2_g, ln2_b):
    bf16 = ml_dtypes.bfloat16
    f32 = np.float32
    x = np.asarray(x, f32)
    W_attn = np.asarray(W_attn, f32)
    Wq = W_attn[:, 0:C] * (1.0 / np.sqrt(D))
    Wk, Wv = W_attn[:, C:2 * C], W_attn[:, 2 * C:3 * C]

    def lhs_tiles(W, nout):
        # [C, nout*128] -> [nout, 128p, NKC, 128m]
        return np.ascontiguousarray(
            np.asarray(W, f32).reshape(NKC, 128, nout, 128)
            .transpose(2, 1, 0, 3)).astype(bf16)

    wq = lhs_tiles(Wq, 8)
    wk = lhs_tiles(Wk, 8)
    wv = np.ascontiguousarray(
        np.asarray(Wv, f32).reshape(NKC, 128, 2, 512)
        .transpose(2, 1, 0, 3)).astype(bf16)
    wp = lhs_tiles(W_proj, 8)
    wf1 = lhs_tiles(W_fc1, NFFC)
    wf2 = np.ascontiguousarray(
        np.asarray(W_fc2, f32).reshape(NFFC, 128, NKC, 128)
        .transpose(2, 1, 0, 3)).astype(bf16)

    def vec(v, nk):
        return np.ascontiguousarray(np.asarray(v, f32).reshape(nk, 128).T)

    # triangular diagonal mask: [128 kv-in-chunk, 2 chunk-in-pair, 256 q]
    kvp = np.arange(128)
    qp = np.arange(256)
    mtri = np.zeros((128, 2, 256), np.float32)
    for cc in range(2):
        mtri[:, cc, :] = ((kvp[:, None] + 128 * cc) <= qp[None, :])
    mtri = mtri.astype(bf16)

    shared = {
        "wq": wq, "wk": wk, "wv": wv, "wp": wp, "wf1": wf1, "wf2": wf2,
        "g1": vec(ln1_g, NKC), "b1": vec(ln1_b, NKC),
        "g2": vec(ln2_g, NKC), "b2": vec(ln2_b, NKC),
        "bp": vec(b_proj, NKC), "bf1": vec(b_fc1, NFFC),
        "bf2": vec(b_fc2, NKC), "mtri": mtri,
    }

    in_maps = []
    for c in range(NC):
        b, r = c // 2, c % 2
        order = _core_token_order(r)
        xs = x[b][order]                      # [T, C] core token order
        xt = np.ascontiguousarray(
            xs.T.reshape(NKC, 128, T).transpose(1, 0, 2)).astype(bf16)
        # partner-pair mask value per block: ones iff (b odd) xor r
        mdep = np.zeros((128, 4), np.float32)
        for blk in range(4):
            ones = (blk % 2 == 1) if r == 0 else (blk % 2 == 0)
            mdep[:, blk] = 1.0 if ones else 0.0
        d = {"xt": xt, "mdep": mdep}
        d.update(shared)
        in_maps.append(d)
    return in_maps


def kernel(x, W_attn, W_proj, b_proj, W_fc1, b_fc1, W_fc2, b_fc2,
           ln1_g, ln1_b, ln2_g, ln2_b):
    runner = _get_runner()
    in_maps = _prep_in_maps(x, W_attn, W_proj, b_proj, W_fc1, b_fc1,
                            W_fc2, b_fc2, ln1_g, ln1_b, ln2_g, ln2_b)
    runner.put_inputs(in_maps)
    results = runner.run()
    out = np.empty((B, T, C), np.float32)
    for c in range(NC):
        b, r = c // 2, c % 2
        ot = results[c]["out"]                # [128, NKC, OWN]
        feat = ot.transpose(1, 0, 2).reshape(C, OWN)
        out[b, _q_token_sel(r), :] = feat.T
    return out


# revision 19
# speedup vs baseline: 2.9236x; 2.9236x over previous
"""Dense transformer block (B=4, T=2048, C=1024, H=16, FF=4096) on 8
Trainium2 NeuronCores.

Sharding: sequence-parallel, zero collectives. Core c handles batch
b = c // 2 and a zigzag set of 1024 query tokens (r = c % 2): 256-token
global blocks {0,3,4,7} for r=0, {1,2,5,6} for r=1. The host permutes
each core's token axis so its OWN query tokens occupy columns [0:1024)
(in block order) and the partner's tokens columns [1024:2048). K/V/LN1
are computed for all 2048 tokens (redundantly within a pair), so no
cross-core communication is needed. Causality is enforced with a
shared triangular diagonal mask plus per-core 0/1 block masks (input
data), which keeps the single SPMD program uniform across cores.

v2: all-bf16 data path (fp32 PSUM accumulation and statistics), no
DRAM bounces (K/V/Q/y stay in SBUF; odd-head y merges via SBUF->SBUF
DMA partition shift), 256-token q-blocks with exact kv-chunk coverage,
batched exp over 2-chunk score groups, ACT Reciprocal/Rsqrt for
softmax/LN, GPSIMD partition_broadcast for row broadcasts, and
attention interleaved with the first FFN half to keep TensorE dense.
"""
import numpy as np
import ml_dtypes

B, T, C = 4, 2048, 1024
H, D, FF = 16, 64, 4096
NC = 8
NKC = C // 128     # 8 feature chunks
NFFC = FF // 128   # 32
NVCH = T // 128    # 16 kv chunks
OWN = 1024         # own query tokens per core
EPS = 1e-5

_STATE = {}

# kv chunk production order: block b needs own chunks [0:2b+2) and
# partner chunks [8:8+2b+2)
VCH_ORDER = [0, 1, 8, 9, 2, 3, 10, 11, 4, 5, 12, 13, 6, 7, 14, 15]


def _build_program():
    import concourse.bacc as bacc
    import concourse.mybir as mybir
    from concourse.tile import TileContext

    F32 = mybir.dt.float32
    BF16 = mybir.dt.bfloat16
    AF = mybir.ActivationFunctionType
    OP = mybir.AluOpType

    nc = bacc.Bacc("TRN2", target_bir_lowering=False, debug=False,
                   num_devices=NC)

    xt_d = nc.dram_tensor("xt", [128, NKC, T], BF16, kind="ExternalInput")
    wq_d = nc.dram_tensor("wq", [8, 128, NKC, 128], BF16, kind="ExternalInput")
    wk_d = nc.dram_tensor("wk", [8, 128, NKC, 128], BF16, kind="ExternalInput")
    wv_d = nc.dram_tensor("wv", [2, 128, NKC, 512], BF16, kind="ExternalInput")
    wp_d = nc.dram_tensor("wp", [8, 128, NKC, 128], BF16, kind="ExternalInput")
    wf1_d = nc.dram_tensor("wf1", [NFFC, 128, NKC, 128], BF16,
                           kind="ExternalInput")
    wf2_d = nc.dram_tensor("wf2", [NKC, 128, NFFC, 128], BF16,
                           kind="ExternalInput")
    g1_d = nc.dram_tensor("g1", [128, NKC], F32, kind="ExternalInput")
    b1_d = nc.dram_tensor("b1", [128, NKC], F32, kind="ExternalInput")
    g2_d = nc.dram_tensor("g2", [128, NKC], F32, kind="ExternalInput")
    b2_d = nc.dram_tensor("b2", [128, NKC], F32, kind="ExternalInput")
    bp_d = nc.dram_tensor("bp", [128, NKC], F32, kind="ExternalInput")
    bf1_d = nc.dram_tensor("bf1", [128, NFFC], F32, kind="ExternalInput")
    bf2_d = nc.dram_tensor("bf2", [128, NKC], F32, kind="ExternalInput")
    # triangular diagonal mask (same on all cores): [128 kv, 2 chunks, 256 q]
    mtri_d = nc.dram_tensor("mtri", [128, 2, 256], BF16, kind="ExternalInput")
    # per-block partner-pair mask value (0.0 or 1.0), per core
    mdep_d = nc.dram_tensor("mdep", [128, 4], F32, kind="ExternalInput")
    out_d = nc.dram_tensor("out", [128, NKC, OWN], F32, kind="ExternalOutput")
    import os as _os
    DBG = _os.environ.get("BASS_DEBUG_TAPS") == "1"
    if DBG:
        dbg_lnx = nc.dram_tensor("dbg_lnx", [128, NKC, T], BF16,
                                 kind="ExternalOutput")
        dbg_q = nc.dram_tensor("dbg_q", [128, NKC, OWN], BF16,
                               kind="ExternalOutput")
        dbg_k = nc.dram_tensor("dbg_k", [128, NKC, T], BF16,
                               kind="ExternalOutput")
        dbg_v = nc.dram_tensor("dbg_v", [128, NVCH, H, 65], BF16,
                               kind="ExternalOutput")
        dbg_y = nc.dram_tensor("dbg_y", [128, NKC, OWN], BF16,
                               kind="ExternalOutput")
        dbg_x2 = nc.dram_tensor("dbg_x2", [128, NKC, OWN], BF16,
                                kind="ExternalOutput")
        dbg_ln2 = nc.dram_tensor("dbg_ln2", [128, NKC, OWN], BF16,
                                 kind="ExternalOutput")

    def mm(ps, lhsT, rhs, start, stop, **kw):
        nc.tensor.matmul(ps, lhsT, rhs, start=start, stop=stop, **kw)

    with TileContext(nc, pool_alloc_mode="queue") as tc:
        consts_cm = tc.tile_pool(name="consts", bufs=1)
        consts = consts_cm.__enter__()

        ones128 = consts.tile([128, 1], BF16)
        nc.vector.memset(ones128, 1.0)
        eps_t = consts.tile([1, 1], F32)
        nc.vector.memset(eps_t, EPS)
        g1t = consts.tile([128, NKC], F32)
        nc.sync.dma_start(out=g1t, in_=g1_d[:, :])
        b1t = consts.tile([128, NKC], F32)
        nc.sync.dma_start(out=b1t, in_=b1_d[:, :])
        g2t = consts.tile([128, NKC], F32)
        nc.sync.dma_start(out=g2t, in_=g2_d[:, :])
        b2t = consts.tile([128, NKC], F32)
        nc.sync.dma_start(out=b2t, in_=b2_d[:, :])
        bpt = consts.tile([128, NKC], F32)
        nc.sync.dma_start(out=bpt, in_=bp_d[:, :])
        bf1t = consts.tile([128, NFFC], F32)
        nc.sync.dma_start(out=bf1t, in_=bf1_d[:, :])
        bf2t = consts.tile([128, NKC], F32)
        nc.sync.dma_start(out=bf2t, in_=bf2_d[:, :])
        mtri = consts.tile([128, 2, 256], BF16)
        nc.sync.dma_start(out=mtri, in_=mtri_d[:, :, :])
        mdep = consts.tile([128, 4], F32)
        nc.sync.dma_start(out=mdep, in_=mdep_d[:, :])

        # ---------------- layer norm over feature dim (bf16) -------------
        def layer_norm(src, dst, gt, bt, psum, work):
            """src [128, NKC, 512] bf16 view; dst same-shape bf16 view.
            Stats via PE ones-matmuls, rstd = exp(-0.5*ln(var+eps)) on ACT
            (Ln/Exp share one table set with attention Exp), broadcasts on
            GPSIMD, normalize on DVE."""
            ps_s = psum.tile([128, 512], F32, tag="mm")
            for k in range(NKC):
                mm(ps_s[0:1, :], ones128, src[:, k, :], k == 0, k == NKC - 1)
            sqs = []
            for k in range(NKC):
                sq = work.tile([128, 512], BF16, tag="sq", bufs=3)
                nc.scalar.activation(out=sq, in_=src[:, k, :], func=AF.Square)
                sqs.append(sq)
            ps_q = psum.tile([128, 512], F32, tag="mm")
            for k in range(NKC):
                mm(ps_q[0:1, :], ones128, sqs[k], k == 0, k == NKC - 1)
            mu_f = work.tile([1, 512], F32, tag="st", bufs=4)
            nc.vector.tensor_scalar_mul(out=mu_f, in0=ps_s[0:1, :],
                                        scalar1=1.0 / C)
            mu2 = work.tile([1, 512], F32, tag="st", bufs=4)
            nc.vector.tensor_mul(out=mu2, in0=mu_f, in1=mu_f)
            var = work.tile([1, 512], F32, tag="st", bufs=4)
            nc.vector.scalar_tensor_tensor(
                out=var, in0=ps_q[0:1, :], scalar=1.0 / C, in1=mu2,
                op0=OP.mult, op1=OP.subtract)
            lnv = work.tile([1, 512], F32, tag="st", bufs=4)
            nc.scalar.activation(out=lnv, in_=var, func=AF.Ln,
                                 bias=eps_t, scale=1.0)
            rstd = work.tile([1, 512], BF16, tag="sv", bufs=3)
            with nc.allow_low_precision(reason="bf16 rstd"):
                nc.scalar.activation(out=rstd, in_=lnv, func=AF.Exp,
                                     scale=-0.5)
            mu_bf = work.tile([1, 512], BF16, tag="sv", bufs=3)
            nc.vector.tensor_copy(out=mu_bf, in_=mu_f)
            mu_b = work.tile([128, 512], BF16, tag="bc", bufs=3)
            nc.gpsimd.partition_broadcast(mu_b, mu_bf)
            rs_b = work.tile([128, 512], BF16, tag="bc", bufs=3)
            nc.gpsimd.partition_broadcast(rs_b, rstd)
            for k in range(NKC):
                t1 = work.tile([128, 512], BF16, tag="tt", bufs=4)
                nc.vector.tensor_sub(out=t1, in0=src[:, k, :], in1=mu_b)
                t2 = work.tile([128, 512], BF16, tag="tt", bufs=4)
                nc.vector.tensor_mul(out=t2, in0=t1, in1=rs_b)
                nc.vector.tensor_scalar(
                    out=dst[:, k, :], in0=t2,
                    scalar1=gt[:, k:k + 1], scalar2=bt[:, k:k + 1],
                    op0=OP.mult, op1=OP.add)

        # ======== Phase 1: LN1 over all 2048 tokens ========
        big_cm = tc.tile_pool(name="big", bufs=1)
        big = big_cm.__enter__()
        lnw_cm = tc.tile_pool(name="lnw", bufs=1)
        lnw = lnw_cm.__enter__()
        lnx_cm = tc.tile_pool(name="lnxp", bufs=1)
        lnxp = lnx_cm.__enter__()
        lnx = lnxp.tile([128, NKC, T], BF16)

        xt_cm = tc.tile_pool(name="xtp", bufs=2)
        xtp = xt_cm.__enter__()
        ps1_cm = tc.tile_pool(name="ln1ps", bufs=4, space="PSUM")
        ps1 = ps1_cm.__enter__()
        for tb in range(4):
            sl = slice(tb * 512, (tb + 1) * 512)
            xtb = xtp.tile([128, NKC, 512], BF16, tag="xtb")
            nc.sync.dma_start(out=xtb, in_=xt_d[:, :, sl])
            layer_norm(xtb, lnx[:, :, sl], g1t, b1t, ps1, lnw)
        ps1_cm.__exit__(None, None, None)
        xt_cm.__exit__(None, None, None)

        # ======== Phase 2: QKV projections (SBUF-resident outputs) =======
        qt = big.tile([128, NKC, OWN], BF16, tag="q")
        kt = big.tile([128, NKC, T], BF16, tag="k")
        vt = big.tile([128, NVCH, H, 65], BF16, tag="v")
        nc.vector.memset(vt[:, :, :, 64:65], 1.0)

        qw_cm = tc.tile_pool(name="qkvw", bufs=3)
        qw = qw_cm.__enter__()
        wv_cm = tc.tile_pool(name="wvp", bufs=2)
        wvp = wv_cm.__enter__()
        psq_cm = tc.tile_pool(name="qkvps", bufs=4, space="PSUM")
        psq = psq_cm.__enter__()

        # Q (own tokens only; 1/sqrt(D) folded into wq host-side)
        for oc in range(8):
            wt = qw.tile([128, NKC, 128], BF16, tag="w")
            nc.sync.dma_start(out=wt, in_=wq_d[oc])
            for tb in range(2):
                sl = slice(tb * 512, (tb + 1) * 512)
                ps = psq.tile([128, 512], F32, tag="mm")
                for k in range(NKC):
                    mm(ps, wt[:, k, :], lnx[:, k, sl], k == 0, k == NKC - 1)
                nc.scalar.activation(out=qt[:, oc, sl], in_=ps, func=AF.Copy)
        # K (all tokens)
        for oc in range(8):
            wt = qw.tile([128, NKC, 128], BF16, tag="w")
            nc.sync.dma_start(out=wt, in_=wk_d[oc])
            for tb in range(4):
                sl = slice(tb * 512, (tb + 1) * 512)
                ps = psq.tile([128, 512], F32, tag="mm")
                for k in range(NKC):
                    mm(ps, wt[:, k, :], lnx[:, k, sl], k == 0, k == NKC - 1)
                nc.scalar.activation(out=kt[:, oc, sl], in_=ps, func=AF.Copy)
        # V (all tokens; natural [token, feature] layout, chunk order by
        # attention need)
        wvt = []
        for g in range(2):
            wvg = wvp.tile([128, NKC, 512], BF16, tag="wv")
            nc.sync.dma_start(out=wvg, in_=wv_d[g])
            wvt.append(wvg)
        for cch in VCH_ORDER:
            for g in range(2):
                ps = psq.tile([128, 512], F32, tag="mm")
                for k in range(NKC):
                    mm(ps, lnx[:, k, cch * 128:(cch + 1) * 128], wvt[g][:, k, :],
                       k == 0, k == NKC - 1)
                nc.scalar.activation(
                    out=vt[:, cch, g * 8:(g + 1) * 8, 0:64], in_=ps,
                    func=AF.Copy)
        psq_cm.__exit__(None, None, None)
        wv_cm.__exit__(None, None, None)
        qw_cm.__exit__(None, None, None)
        lnx_cm.__exit__(None, None, None)

        # ======== pooled tiles for the back half (opened before the
        # attention pools so the attention pools can close mid-stream
        # in LIFO order) ========
        act_cm = tc.tile_pool(name="actp", bufs=1)
        actp = act_cm.__enter__()
        ytp = actp
        x2p = actp
        ln2p = actp
        m1_cm = tc.tile_pool(name="m1p", bufs=1)
        m1p = m1_cm.__enter__()
        tail_cm = tc.tile_pool(name="tailp", bufs=1)
        tailp = tail_cm.__enter__()
        gps_cm = tc.tile_pool(name="gps", bufs=2, space="PSUM")
        gps = gps_cm.__enter__()

        # ======== attention machinery ========
        attw_cm = tc.tile_pool(name="attw", bufs=1)
        attw = attw_cm.__enter__()
        ptmp = attw
        sps_cm = tc.tile_pool(name="attps", bufs=2, space="PSUM")
        sps = sps_cm.__enter__()
        yps_cm = tc.tile_pool(name="attpy", bufs=2, space="PSUM")
        yps = yps_cm.__enter__()

        yt_tiles = {}

        def attn(hp, b):
            """Head pair hp (heads 2hp, 2hp+1), own q block b (256 toks)."""
            qsl = slice(b * 256, (b + 1) * 256)
            chunks = list(range(0, 2 * b + 2)) + list(range(8, 8 + 2 * b + 2))
            groups = [(chunks[2 * i], chunks[2 * i + 1])
                      for i in range(len(chunks) // 2)]
            ng = len(groups)
            yt = yt_tiles[b // 2]
            bsl = slice((b % 2) * 256, (b % 2) * 256 + 256)
            ps_y = yps.tile([65, 2, 256], F32, tag="y")
            first = True
            for gi, (c0, c1) in enumerate(groups):
                # layout [128, head, chunk, 256]: the two concurrently
                # row-tiled score matmuls (head 0/1) land in different
                # PSUM banks (concurrent PE writes to one bank are unsafe)
                ps_s = sps.tile([128, 2, 2, 256], F32, tag="s")
                for ji, ch in enumerate((c0, c1)):
                    csl = slice(ch * 128, (ch + 1) * 128)
                    mm(ps_s[:, 0, ji, :], kt[0:64, hp, csl],
                       qt[0:64, hp, qsl], True, True)
                    mm(ps_s[:, 1, ji, :], kt[64:128, hp, csl],
                       qt[64:128, hp, qsl], True, True)
                ptm = ptmp.tile([128, 2, 2, 256], BF16, tag="ptm", bufs=3)
                nc.scalar.activation(out=ptm, in_=ps_s, func=AF.Exp)
                if c0 == 2 * b:        # diagonal own pair -> triangular mask
                    for ji in range(2):
                        nc.vector.tensor_mul(
                            out=ptm[:, :, ji, :], in0=ptm[:, :, ji, :],
                            in1=mtri[:, ji:ji + 1, :]
                            .broadcast_to([128, 2, 256]))
                elif c0 == 8 + 2 * b:  # partner diagonal pair -> 0/1 scalar
                    nc.vector.tensor_scalar_mul(
                        out=ptm, in0=ptm, scalar1=mdep[:, b:b + 1])
                for ji, ch in enumerate((c0, c1)):
                    for h in range(2):
                        mm(ps_y[:, h, :], vt[:, ch, 2 * hp + h, 0:65],
                           ptm[:, h, ji, :], first,
                           (gi == ng - 1) and (ji == 1) and (h == 1),
                           skip_group_check=True)
                        first = False
            # softmax tail: 1/den = exp(-ln(den)) on ACT (same table set
            # as the attention Exp -> no ACT_TABLE_LOAD switching), then
            # GPSIMD partition broadcast and DVE scaling.
            lnden = attw.tile([1, 2, 256], F32, tag="lnden", bufs=2)
            nc.scalar.activation(out=lnden, in_=ps_y[64:65, :, :], func=AF.Ln)
            rd = attw.tile([1, 2, 256], BF16, tag="rd", bufs=2)
            with nc.allow_low_precision(reason="bf16 softmax denom recip"):
                nc.scalar.activation(out=rd, in_=lnden, func=AF.Exp,
                                     scale=-1.0)
            rb = attw.tile([64, 2, 256], BF16, tag="rb", bufs=2)
            nc.gpsimd.partition_broadcast(rb, rd)
            nc.vector.tensor_mul(out=yt[0:64, hp, bsl],
                                 in0=ps_y[0:64, 0, :], in1=rb[:, 0, :])
            ytmp = attw.tile([64, 256], BF16, tag="ytmp", bufs=2)
            nc.vector.tensor_mul(out=ytmp, in0=ps_y[0:64, 1, :],
                                 in1=rb[:, 1, :])
            nc.sync.dma_start(out=yt[64:128, hp, bsl], in_=ytmp)

        x2_tiles = {}
        ln2_tiles = {}
        m1_tiles = {}

        def proj_ln2(h):
            """proj + residual + LN2 for own-token half h (512 tokens)."""
            hsl = slice(h * 512, (h + 1) * 512)
            yt = yt_tiles[h]
            x2t = x2p.tile([128, NKC, 512], BF16, tag="x2", name=f"x2h{h}", bufs=2)
            x2_tiles[h] = x2t
            for oc in range(8):
                wt = tailp.tile([128, NKC, 128], BF16, tag="w", bufs=3)
                nc.sync.dma_start(out=wt, in_=wp_d[oc])
                xq = tailp.tile([128, 512], BF16, tag="xq", bufs=3)
                nc.sync.dma_start(out=xq, in_=xt_d[:, oc, hsl])
                ps = gps.tile([128, 512], F32, tag="mm")
                for k in range(NKC):
                    mm(ps, wt[:, k, :], yt[:, k, :], k == 0, k == NKC - 1)
                nc.vector.scalar_tensor_tensor(
                    out=x2t[:, oc, :], in0=ps, scalar=bpt[:, oc:oc + 1],
                    in1=xq, op0=OP.add, op1=OP.add)
            ln2t = ln2p.tile([128, NKC, 512], BF16, tag="ln2", name=f"ln2h{h}", bufs=1)
            ln2_tiles[h] = ln2t
            layer_norm(x2t, ln2t, g2t, b2t, gps, lnw)

        def fc1(h, ffc):
            wt = tailp.tile([128, NKC, 128], BF16, tag="w", bufs=3)
            nc.sync.dma_start(out=wt, in_=wf1_d[ffc])
            ps = gps.tile([128, 512], F32, tag="mm")
            for k in range(NKC):
                mm(ps, wt[:, k, :], ln2_tiles[h][:, k, :], k == 0, k == NKC - 1)
            nc.vector.tensor_scalar(
                out=m1_tiles[h][:, ffc, :], in0=ps,
                scalar1=bf1t[:, ffc:ffc + 1], scalar2=0.0,
                op0=OP.add, op1=OP.max)

        def fc2(h, oc, psum):
            hsl = slice(h * 512, (h + 1) * 512)
            ps = psum.tile([128, 512], F32, tag="mm")
            for half in range(2):
                wt2 = tailp.tile([128, 16, 128], BF16, tag="w2", bufs=2)
                nc.sync.dma_start(out=wt2,
                                  in_=wf2_d[oc][:, half * 16:half * 16 + 16, :])
                for j in range(16):
                    k = half * 16 + j
                    mm(ps, wt2[:, j, :], m1_tiles[h][:, k, :],
                       k == 0, k == NFFC - 1)
            ot = tailp.tile([128, 512], F32, tag="ot", bufs=2)
            nc.vector.scalar_tensor_tensor(
                out=ot, in0=ps, scalar=bf2t[:, oc:oc + 1],
                in1=x2_tiles[h][:, oc, :], op0=OP.add, op1=OP.add)
            nc.sync.dma_start(out=out_d[:, oc, hsl], in_=ot)

        # ======== Phase 3: attention blocks 0,1 then proj/LN2 half 0 =====
        yt_tiles[0] = ytp.tile([128, NKC, 512], BF16, tag="yt", name="yt0", bufs=1)
        for hp in range(8):
            attn(hp, 0)
            attn(hp, 1)
        proj_ln2(0)
        m1_tiles[0] = m1p.tile([128, NFFC, 512], BF16, tag="m1", name="m1h0")

        # ======== Phase 5: attention blocks 2,3 interleaved with FC1(h0) ==
        yt_tiles[1] = ytp.tile([128, NKC, 512], BF16, tag="yt", name="yt1", bufs=1)
        for hp in range(8):
            attn(hp, 2)
            fc1(0, 4 * hp + 0)
            fc1(0, 4 * hp + 1)
            attn(hp, 3)
            fc1(0, 4 * hp + 2)
            fc1(0, 4 * hp + 3)

        # ======== Phase 6: FC2 half 0; proj/LN2 half 1 ========
        for oc in range(4):
            fc2(0, oc, gps)
        proj_ln2(1)
        for oc in range(4, 8):
            fc2(0, oc, gps)

        if DBG:
            nc.sync.dma_start(out=dbg_lnx[:, :, :], in_=lnx)
            nc.sync.dma_start(out=dbg_q[:, :, :], in_=qt)
            nc.sync.dma_start(out=dbg_k[:, :, :], in_=kt)
            nc.sync.dma_start(out=dbg_v[:, :, :, :], in_=vt)
            for h in range(2):
                hsl = slice(h * 512, (h + 1) * 512)
                nc.sync.dma_start(out=dbg_y[:, :, hsl], in_=yt_tiles[h])
                nc.sync.dma_start(out=dbg_x2[:, :, hsl], in_=x2_tiles[h])
                nc.sync.dma_start(out=dbg_ln2[:, :, hsl], in_=ln2_tiles[h])

        # attention pools closed -> free PSUM for the tail
        yps_cm.__exit__(None, None, None)
        sps_cm.__exit__(None, None, None)
        attw_cm.__exit__(None, None, None)

        tl_cm = tc.tile_pool(name="tailps", bufs=4, space="PSUM")
        tlps = tl_cm.__enter__()

        # ======== Phase 7: FFN half 1 ========
        m1_tiles[1] = m1p.tile([128, NFFC, 512], BF16, tag="m1", name="m1h1")
        for ffc in range(NFFC):
            fc1(1, ffc)
        for oc in range(8):
            fc2(1, oc, tlps)

        tl_cm.__exit__(None, None, None)
        gps_cm.__exit__(None, None, None)
        tail_cm.__exit__(None, None, None)
        m1_cm.__exit__(None, None, None)
        act_cm.__exit__(None, None, None)
        lnw_cm.__exit__(None, None, None)
        big_cm.__exit__(None, None, None)
        consts_cm.__exit__(None, None, None)

    nc.compile()
    return nc


class _SpmdRunner:
    def __init__(self, nc, n_cores=NC):
        import jax
        from jax.sharding import Mesh, PartitionSpec
        from jax.experimental.shard_map import shard_map
        import concourse.mybir as mybir
        from concourse import bass2jax
        bass2jax.install_neuronx_cc_hook()
        self.jax = jax
        self.n_cores = n_cores
        partition_name = (
            nc.partition_id_tensor.name if nc.partition_id_tensor else None)
        in_names, out_names, out_avals = [], [], []
        for alloc in nc.m.functions[0].allocations:
            if not isinstance(alloc, mybir.MemoryLocationSet):
                continue
            name = alloc.memorylocations[0].name
            if alloc.kind == "ExternalInput":
                if name != partition_name:
                    in_names.append(name)
            elif alloc.kind == "ExternalOutput":
                out_names.append(name)
                out_avals.append(jax.core.ShapedArray(
                    tuple(alloc.tensor_shape), mybir.dt.np(alloc.dtype)))
        self.in_names = in_names
        self.out_names = out_names
        self.out_avals = out_avals
        all_in = in_names + out_names
        if partition_name is not None:
            all_in.append(partition_name)

        def _body(*args):
            operands = list(args)
            if partition_name is not None:
                operands.append(bass2jax.partition_id_tensor())
            outs = bass2jax._bass_exec_p.bind(
                *operands, out_avals=tuple(out_avals),
                in_names=tuple(all_in), out_names=tuple(out_names),
                lowering_input_output_aliases=(),
                sim_require_finite=True, sim_require_nnan=True, nc=nc)
            return tuple(outs)

        import os as _os
        if _os.environ.get("BASS_SIM") == "1":
            devices = jax.devices("cpu")[:n_cores]
        else:
            devices = jax.devices()[:n_cores]
        self.mesh = Mesh(np.asarray(devices), ("core",))
        n_io = len(in_names) + len(out_names)
        self.fn = jax.jit(
            shard_map(_body, mesh=self.mesh,
                      in_specs=(PartitionSpec("core"),) * n_io,
                      out_specs=(PartitionSpec("core"),) * len(out_names),
                      check_rep=False),
            keep_unused=True)
        self._dev_in = None

    def put_inputs(self, in_maps):
        from jax.sharding import NamedSharding, PartitionSpec
        jax = self.jax
        sh = NamedSharding(self.mesh, PartitionSpec("core"))
        concat = []
        for name in self.in_names:
            arrs = [np.asarray(in_maps[c][name]) for c in range(self.n_cores)]
            concat.append(jax.device_put(np.concatenate(arrs, axis=0), sh))
        for av in self.out_avals:
            z = np.zeros((self.n_cores * av.shape[0], *av.shape[1:]), av.dtype)
            concat.append(jax.device_put(z, sh))
        self._dev_in = concat

    def run(self):
        jax = self.jax
        outs = self.fn(*self._dev_in)
        jax.block_until_ready(outs)
        results = []
        for c in range(self.n_cores):
            d = {}
            for i, name in enumerate(self.out_names):
                av = self.out_avals[i]
                d[name] = np.asarray(outs[i]).reshape(
                    self.n_cores, *av.shape)[c]
            results.append(d)
        return results

    def time_exec(self, warmup=3, m1=4, m2=12, reps=3, trials=6):
        """Estimate per-call device time by dispatching bursts of m1 and
        m2 back-to-back calls and differencing, which cancels the
        constant dispatch/RTT overhead of the axon tunnel."""
        import time
        jax = self.jax
        for _ in range(warmup):
            jax.block_until_ready(self.fn(*self._dev_in))

        def burst(m):
            t0 = time.perf_counter()
            outs = None
            for _ in range(m):
                outs = self.fn(*self._dev_in)
            jax.block_until_ready(outs)
            return time.perf_counter() - t0

        t1s, t2s = [], []
        for _ in range(trials):
            for _ in range(reps):
                t1s.append(burst(m1))
                t2s.append(burst(m2))
        return (min(t2s) - min(t1s)) / (m2 - m1)


def _get_runner():
    if "runner" not in _STATE:
        nc = _build_program()
        _STATE["runner"] = _SpmdRunner(nc)
    return _STATE["runner"]


def _q_token_sel(r):
    """256-token global blocks {0,3,4,7} for r=0, {1,2,5,6} for r=1."""
    if r == 0:
        return np.concatenate([np.arange(0, 256), np.arange(768, 1280),
                               np.arange(1792, 2048)])
    return np.concatenate([np.arange(256, 768), np.arange(1280, 1792)])


def _core_token_order(r):
    own = _q_token_sel(r)
    partner = np.setdiff1d(np.arange(T), own)
    return np.concatenate([own, partner])


def _prep_in_maps(x, W_attn, W_proj, b_proj, W_fc1, b_fc1, W_fc2, b_fc2,
                  ln1_g, ln1_b, ln2_g, ln2_b):
    bf16 = ml_dtypes.bfloat16
    f32 = np.float32
    x = np.asarray(x, f32)
    W_attn = np.asarray(W_attn, f32)
    Wq = W_attn[:, 0:C] * (1.0 / np.sqrt(D))
    Wk, Wv = W_attn[:, C:2 * C], W_attn[:, 2 * C:3 * C]

    def lhs_tiles(W, nout):
        # [C, nout*128] -> [nout, 128p, NKC, 128m]
        return np.ascontiguousarray(
            np.asarray(W, f32).reshape(NKC, 128, nout, 128)
            .transpose(2, 1, 0, 3)).astype(bf16)

    wq = lhs_tiles(Wq, 8)
    wk = lhs_tiles(Wk, 8)
    wv = np.ascontiguousarray(
        np.asarray(Wv, f32).reshape(NKC, 128, 2, 512)
        .transpose(2, 1, 0, 3)).astype(bf16)
    wp = lhs_tiles(W_proj, 8)
    wf1 = lhs_tiles(W_fc1, NFFC)
    wf2 = np.ascontiguousarray(
        np.asarray(W_fc2, f32).reshape(NFFC, 128, NKC, 128)
        .transpose(2, 1, 0, 3)).astype(bf16)

    def vec(v, nk):
        return np.ascontiguousarray(np.asarray(v, f32).reshape(nk, 128).T)

    # triangular diagonal mask: [128 kv-in-chunk, 2 chunk-in-pair, 256 q]
    kvp = np.arange(128)
    qp = np.arange(256)
    mtri = np.zeros((128, 2, 256), np.float32)
    for cc in range(2):
        mtri[:, cc, :] = ((kvp[:, None] + 128 * cc) <= qp[None, :])
    mtri = mtri.astype(bf16)

    shared = {
        "wq": wq, "wk": wk, "wv": wv, "wp": wp, "wf1": wf1, "wf2": wf2,
        "g1": vec(ln1_g, NKC), "b1": vec(ln1_b, NKC),
        "g2": vec(ln2_g, NKC), "b2": vec(ln2_b, NKC),
        "bp": vec(b_proj, NKC), "bf1": vec(b_fc1, NFFC),
        "bf2": vec(b_fc2, NKC), "mtri": mtri,
    }

    in_maps = []
    for c in range(NC):
        b, r = c // 2, c % 2
        order = _core_token_order(r)
        xs = x[b][order]                      # [T, C] core token order
        xt = np.ascontiguousarray(
            xs.T.reshape(NKC, 128, T).transpose(1, 0, 2)).astype(bf16)
        # partner-pair mask value per block: ones iff (b odd) xor r
        mdep = np.zeros((128, 4), np.float32)
        for blk in range(4):
            ones = (blk % 2 == 1) if r == 0 else (blk % 2 == 0)
            mdep[:, blk] = 1.0 if ones else 0.0
        d = {"xt": xt, "mdep": mdep}
        d.update(shared)
        in_maps.append(d)
    return in_maps


def kernel(x, W_attn, W_proj, b_proj, W_fc1, b_fc1, W_fc2, b_fc2,
           ln1_g, ln1_b, ln2_g, ln2_b):
    runner = _get_runner()
    in_maps = _prep_in_maps(x, W_attn, W_proj, b_proj, W_fc1, b_fc1,
                            W_fc2, b_fc2, ln1_g, ln1_b, ln2_g, ln2_b)
    runner.put_inputs(in_maps)
    results = runner.run()
    out = np.empty((B, T, C), np.float32)
    for c in range(NC):
        b, r = c // 2, c % 2
        ot = results[c]["out"]                # [128, NKC, OWN]
        feat = ot.transpose(1, 0, 2).reshape(C, OWN)
        out[b, _q_token_sel(r), :] = feat.T
    return out
